# revision 42
# baseline (speedup 1.0000x reference)
"""Trainium2 Bass kernel: DeformableValueAttention (exp-spine schedule).

Full-input contract: kernel(**inputs) takes the unsharded inputs of
reference.setup_inputs() and returns the full [B, C, H, W] output.

Sharding: 8 cores = (batch b, head-group g). Each core computes 4 of the 8
attention heads for ALL 1024 queries of one batch and produces a PARTIAL
[C, N] output (its 4 heads' contribution through Wo, fp16); the host sums
the two partials per batch in fp32.

Schedule ("exp spine"): the ACT engine's 32 Exp tiles (~1.1us each on a
[128,1024] fp32 PSUM -> bf16 pu tile, ~1.3us/unit with issue overhead) are
the hard serial resource (~42us); PE work (~36us at 2.4GHz full p-state)
is packed around it so the spine never stalls:

  A  QT(hp0,qf0) + KT(hp0) chunk0 as soon as their DMA lands (~12us).
  B  hp0 spine, qf-outer (16 units): S-pair -> exp. PE backfill between
     pairs: KT(hp0) JIT chunks, QT(hp0,qf1), QT/KT(hp1), V, Vd (banded
     grid_sample matmul; (1+gamma*sal) folded into gt host-side; a 65th
     ones-column per head gives the softmax denominator for free).
  C  hp1 spine, qf-outer: PE backfill O(hp0) -> norm(hp0) -> O(hp1)
     lag-behind; norm(hp1,qf0) closes mid-C.
  D  last O pair -> Wo(qf0) under norm(hp1,qf1) -> Wo(qf1), per-pt
     pipelined fp16 copies + out DMA.

PSUM (8 banks): tag "ps_s" 2 x [128,1024] (4 banks) double-buffering the
score tiles; tag "ps_acc" 4 x 1-bank slots time-shared in rotation:
projections/V/Vd (B) -> O accumulators [65,512] of one head-pair (C) ->
Wo chunks (D). The emission order is arranged so each slot's previous
occupant is always already drained.

DMA: host pre-tiles every input to a flat [128, X] layout (one descriptor
per partition row). All spine-feeding inputs are issued at t0 and
fair-share the 16 DMA engines; only gt (first consumed ~13us into the
spine) and wo (phase D) are held behind a WAW pre-write on the xkv-c0
transfer. Holding anything the early spine consumes was tried and hurts:
the scheduler interleaves the dependent matmuls into the PE queue, making
held transfers spine-blocking. A dummy Exp at t~1us pre-loads the ACT
function table so the first spine exp doesn't pay the table load.

Engines: PE all matmuls (the two K=64 score matmuls of a pair ride
disjoint PE row groups and overlap); ACT only Exp + output fp16 copies;
DVE PSUM->SBUF casts, reciprocal, norm multiplies; Pool partition
broadcasts and the xkv/gt DMA queue.

Notes on fidelity vs reference.py:
  - P_thermal adds a per-query constant to scores pre-softmax; softmax is
    exactly invariant to that, so it is skipped.
  - All biases in setup_inputs() are zeros; nonzero biases or off-spec
    shapes fall back to a numpy reference implementation.
"""

import sys

import numpy as np
import ml_dtypes

try:
    import concourse.bass as bass  # noqa: F401
except ImportError:  # pragma: no cover - path fallback for bare containers
    sys.path.insert(0, "/opt/trn_rl_repo")
    import concourse.bass as bass  # noqa: F401

import concourse.bacc as bacc
import concourse.tile as tile
from concourse import mybir
from concourse.bass_utils import run_bass_kernel_spmd

B, C, HH, WW = 4, 512, 32, 32
N = HH * WW          # 1024 spatial positions = keys = queries
NH, HD = 8, 64       # total heads, head dim
G = 2                # head groups (cores per batch)
HG = NH // G         # heads per core (4)
CG = HG * HD         # channels per core (256)
P = 128
CT = C // P          # 4 input-channel partition-tiles
NKT = N // P         # 8 key tiles
NCORES = 8
BF16 = mybir.dt.bfloat16
FP16 = mybir.dt.float16
FP32 = mybir.dt.float32
NP_BF16 = ml_dtypes.bfloat16
EXP = mybir.ActivationFunctionType.Exp


# --------------------------------------------------------------------------
# host-side helpers
# --------------------------------------------------------------------------

def _gather_T(offsets_b, salf_b):
    """GT[k, n]: weight of source pixel k in grid-sampled output pixel n,
    with the per-source value modulation salf folded in. fp32 [N, N]."""
    ys = np.linspace(-1.0, 1.0, HH)
    xs = np.linspace(-1.0, 1.0, WW)
    gy, gx = np.meshgrid(ys, xs, indexing="ij")
    x = ((gx + offsets_b[0] / (WW / 2.0) + 1.0) * WW - 1.0) * 0.5
    y = ((gy + offsets_b[1] / (HH / 2.0) + 1.0) * HH - 1.0) * 0.5
    x = np.clip(x, 0.0, WW - 1.0)
    y = np.clip(y, 0.0, HH - 1.0)
    x0 = np.floor(x); y0 = np.floor(y)
    wx = x - x0; wy = y - y0
    x0i = x0.astype(np.int64); y0i = y0.astype(np.int64)
    x1i = np.minimum(x0i + 1, WW - 1); y1i = np.minimum(y0i + 1, HH - 1)
    GT = np.zeros((N, N), np.float32)
    n_idx = np.arange(N)
    for yi, xi, w in ((y0i, x0i, (1 - wx) * (1 - wy)),
                      (y0i, x1i, wx * (1 - wy)),
                      (y1i, x0i, (1 - wx) * wy),
                      (y1i, x1i, wx * wy)):
        np.add.at(GT, ((yi * WW + xi).reshape(-1), n_idx),
                  w.reshape(-1).astype(np.float32))
    GT *= salf_b[:, None]
    return GT


def _reference_numpy(q_feat, kv_feat, offsets, saliency_map, P_thermal,
                     Wq, bq, Wk, bk, Wv, bv, Wo, bo, lambda_p, gamma_val):
    """Plain numpy port of reference.py -- correctness fallback only."""
    Bq, Cq = q_feat.shape[0], q_feat.shape[1]
    Nq = q_feat.shape[2] * q_feat.shape[3]
    qf = q_feat.reshape(Bq, Cq, Nq).transpose(0, 2, 1)
    kf = kv_feat.reshape(Bq, Cq, Nq).transpose(0, 2, 1)

    def heads(x, Wm, bm):
        return (x @ Wm.T + bm).reshape(Bq, Nq, NH, -1).transpose(0, 2, 1, 3)

    Q = heads(qf, Wq, bq)
    K = heads(kf, Wk, bk)
    V = heads(kf, Wv, bv)
    hd = Cq // NH
    attn = np.einsum("bhqd,bhkd->bhqk", Q, K) * (hd ** -0.5)
    attn = attn + float(lambda_p) * P_thermal.reshape(Bq, 1, Nq, 1)
    attn = attn - attn.max(axis=-1, keepdims=True)
    w = np.exp(attn)
    w /= w.sum(axis=-1, keepdims=True)
    Vm = V * (1.0 + float(gamma_val) * saliency_map.reshape(Bq, 1, Nq, 1))
    Vsp = Vm.transpose(0, 2, 1, 3).reshape(Bq, Nq, Cq).transpose(0, 2, 1)
    Vd = np.empty_like(Vsp)
    for b in range(Bq):
        GT = _gather_T(offsets[b], np.ones(Nq, np.float32))
        Vd[b] = Vsp[b] @ GT
    Vdf = Vd.reshape(Bq, Cq, Nq).transpose(0, 2, 1).reshape(Bq, Nq, NH, hd).transpose(0, 2, 1, 3)
    out = np.einsum("bhqk,bhkd->bhqd", w, Vdf)
    out = out.transpose(0, 2, 1, 3).reshape(Bq, Nq, Cq)
    out = out @ Wo.T + bo
    return out.transpose(0, 2, 1).reshape(q_feat.shape).astype(np.float32)


# --------------------------------------------------------------------------
# device program
# --------------------------------------------------------------------------

def _build_program(chunks):
    """chunks: ordered list of (m, k) gather-tile pairs; same for all cores."""
    nch = len(chunks)
    chunks_for_m = {m: [] for m in range(NKT)}
    for idx, (m, k) in enumerate(chunks):
        chunks_for_m[m].append((idx, k))

    nc = bacc.Bacc(None, target_bir_lowering=False, debug=False)
    xq_d = nc.declare_dram_parameter("xq", [P, CT * N], BF16, isOutput=False)
    xkv_d = nc.declare_dram_parameter("xkv", [P, CT * N], BF16,
                                      isOutput=False)
    wq_d = nc.declare_dram_parameter("wqT", [P, CT * CG], BF16,
                                     isOutput=False)
    wk_d = nc.declare_dram_parameter("wkT", [P, CT * CG], BF16,
                                     isOutput=False)
    wv_d = nc.declare_dram_parameter("wvT", [P, CT * CG], BF16,
                                     isOutput=False)
    wo_d = nc.declare_dram_parameter("woT", [P, G * C], BF16, isOutput=False)
    gt_d = nc.declare_dram_parameter("gt", [P, nch * P], BF16,
                                     isOutput=False)
    out_d = nc.declare_dram_parameter("outT", [C, N], FP16, isOutput=True)

    with tile.TileContext(nc) as tc:
        with tc.tile_pool(name="const", bufs=1) as const, \
             tc.tile_pool(name="work", bufs=1) as work, \
             tc.tile_pool(name="pu_pool", bufs=1) as pu_pool, \
             tc.tile_pool(name="sm", bufs=4) as sm, \
             tc.tile_pool(name="psp", bufs=2, space="PSUM") as psp:

            # ---- SBUF input tiles (host pre-tiled to [128, X] layouts) ----
            xq_sb = const.tile([P, CT * N], BF16, name="xq", tag="xq")
            xkv_sb = const.tile([P, CT * N], BF16, name="xkv", tag="xkv")
            wq_sb = const.tile([P, CT * CG], BF16, name="wq", tag="wq")
            wk_sb = const.tile([P, CT * CG], BF16, name="wk", tag="wk")
            wv_sb = const.tile([P, CT * CG], BF16, name="wv", tag="wv")
            gt_w = const.tile([P, nch * P], BF16, name="gtw", tag="gtw")
            wo_sb = const.tile([P, G * C], BF16, name="wo", tag="wo")

            # ---- input DMAs, priority-ordered per queue -------------------
            # sync: xq qf0 then qf1 (QT(hp0,qf0) starts ~3us in); scalar: the
            # weight stack (ACT is idle until the first exp); pool: xkv in
            # 256-col chunks (KT m-chunks go just-in-time), then gt.
            def colchunk(t, lo, hi):
                return t[:].rearrange("p (k n) -> p k n", n=N)[:, :, lo:hi]

            # tiny dummy Exp pre-loads the ACT function table at t~1us so
            # the first spine exp doesn't pay the 1.3us table load
            dumm = sm.tile([1, 2], FP32, name="dumm", tag="dumm", bufs=1)
            nc.vector.memset(dumm[0:1, 0:1], 0.0)
            nc.scalar.activation(out=dumm[0:1, 1:2], in_=dumm[0:1, 0:1],
                                 func=EXP)
            nc.sync.dma_start(out=colchunk(xq_sb, 0, 512),
                              in_=colchunk(xq_d, 0, 512))
            nc.sync.dma_start(out=colchunk(xq_sb, 512, 1024),
                              in_=colchunk(xq_d, 512, 1024))
            for w_d, w_sb in ((wq_d, wq_sb), (wk_d, wk_sb), (wv_d, wv_sb)):
                nc.scalar.dma_start(out=w_sb[:], in_=w_d[:])
            for j in range(4):
                nc.gpsimd.dma_start(
                    out=colchunk(xkv_sb, j * 256, (j + 1) * 256),
                    in_=colchunk(xkv_d, j * 256, (j + 1) * 256))
            # gt (first consumed by Vd, ~13us into the spine) and wo (phase
            # D only) are the only transfers safely gateable: holding them
            # behind a WAW pre-write that depends on the xkv-c0 transfer
            # shrinks the critical DMA wave by 0.9MB without any early
            # spine consumer going head-of-line on a held transfer.
            gate = sm.tile([1, 2], BF16, name="gate", tag="gate", bufs=1)
            nc.gpsimd.tensor_copy(gate[0:1, 0:1], xkv_sb[0:1, 0:1])
            nc.gpsimd.tensor_copy(gt_w[0:1, 0:1], gate[0:1, 0:1])
            nc.gpsimd.tensor_copy(wo_sb[0:1, 0:1], gate[0:1, 0:1])
            nc.gpsimd.dma_start(out=gt_w[:], in_=gt_d[:])
            nc.gpsimd.dma_start(out=wo_sb[:], in_=wo_d[:])

            # ---- SBUF result tiles ----------------------------------------
            pu_buf = pu_pool.tile([P, 32 * N], BF16, name="pu", tag="pu")
            v_buf = work.tile([P, NKT * CG], BF16, name="v", tag="v")
            vd_buf = work.tile([P, NKT * HG * (HD + 1)], BF16, name="vd",
                               tag="vd")
            qt_sb = {hp: work.tile([P, N], BF16, name=f"qt{hp}", tag=f"qt{hp}")
                     for hp in range(G)}
            kt_sb = {hp: work.tile([P, N], BF16, name=f"kt{hp}", tag=f"kt{hp}")
                     for hp in range(G)}
            v_sb = {}
            vd_sb = {}
            o_sb = {hp: work.tile([P, N], BF16, name=f"o{hp}", tag=f"o{hp}")
                    for hp in range(G)}
            pu_tiles = {}
            ps_o = {}
            pu_slot = [0]

            # ---- emission helpers -----------------------------------------
            def emit_qt_chunk(hp, qf):
                # qt_sb[hp][:, qf*512:] = (Wq_hp @ xq)[:, qf half]
                cols = slice(qf * 512, (qf + 1) * 512)
                ps = psp.tile([P, 512], FP32, name=f"psq{hp}{qf}",
                              tag="ps_acc", bufs=4)
                for k in range(CT):
                    nc.tensor.matmul(
                        ps[:],
                        lhsT=wq_sb[:, k * CG + hp * P:k * CG + (hp + 1) * P],
                        rhs=xq_sb[:, k * N + qf * 512:k * N + (qf + 1) * 512],
                        start=(k == 0), stop=(k == CT - 1))
                nc.vector.tensor_copy(qt_sb[hp][:, cols], ps[:])

            def emit_kt_chunk(hp, j):
                # kt_sb[hp][:, j*256:(j+1)*256] (key-tile pair 2j, 2j+1)
                cols = slice(j * 256, (j + 1) * 256)
                ps = psp.tile([P, 256], FP32, name=f"psk{hp}{j}",
                              tag="ps_acc", bufs=4)
                for k in range(CT):
                    nc.tensor.matmul(
                        ps[:],
                        lhsT=wk_sb[:, k * CG + hp * P:k * CG + (hp + 1) * P],
                        rhs=xkv_sb[:, k * N + j * 256:k * N + (j + 1) * 256],
                        start=(k == 0), stop=(k == CT - 1))
                nc.vector.tensor_copy(kt_sb[hp][:, cols], ps[:])

            def emit_v(m):
                ps = psp.tile([P, CG], FP32, name=f"psv{m}", tag="ps_acc",
                              bufs=4)
                for k in range(CT):
                    nc.tensor.matmul(ps[:],
                                     lhsT=xkv_sb[:, k * N + m * P:k * N + (m + 1) * P],
                                     rhs=wv_sb[:, k * CG:(k + 1) * CG],
                                     start=(k == 0), stop=(k == CT - 1))
                tl = v_buf[:, m * CG:(m + 1) * CG]
                nc.vector.tensor_copy(tl, ps[:])
                v_sb[m] = tl

            def emit_vd(m):
                ps = psp.tile([P, CG], FP32, name=f"psvd{m}", tag="ps_acc",
                              bufs=4)
                lst = chunks_for_m[m]
                for j, (idx, k) in enumerate(lst):
                    nc.tensor.matmul(ps[:],
                                     lhsT=gt_w[:, idx * P:(idx + 1) * P],
                                     rhs=v_sb[k],
                                     start=(j == 0), stop=(j == len(lst) - 1))
                w_vd = HG * (HD + 1)
                tl = vd_buf[:, m * w_vd:(m + 1) * w_vd]
                tl3 = tl.rearrange("p (h e) -> p h e", e=HD + 1)
                nc.vector.tensor_copy(
                    tl3[:, :, 0:HD],
                    ps[:].rearrange("p (h e) -> p h e", e=HD))
                nc.vector.memset(tl3[:, :, HD:HD + 1], 1.0)
                vd_sb[m] = tl

            def emit_s(hp, m, qf):
                # scores for both heads of pair hp, key-tile m, query half
                # qf; exp straight off PSUM into a bf16 pu tile.
                kt, qt = kt_sb[hp], qt_sb[hp]
                ps_s = psp.tile([P, N], FP32, name=f"pss{hp}{m}{qf}",
                                tag="ps_s", bufs=2)
                nc.tensor.matmul(
                    ps_s[:, 0:512],
                    lhsT=kt[0:HD, m * P:(m + 1) * P],
                    rhs=qt[0:HD, qf * 512:(qf + 1) * 512],
                    start=True, stop=True)
                nc.tensor.matmul(
                    ps_s[:, 512:1024],
                    lhsT=kt[HD:P, m * P:(m + 1) * P],
                    rhs=qt[HD:P, qf * 512:(qf + 1) * 512],
                    start=True, stop=True)
                s = pu_slot[0] % 32
                pu_slot[0] += 1
                pu = pu_buf[:, s * N:(s + 1) * N]
                nc.scalar.activation(out=pu, in_=ps_s[:], func=EXP)
                pu_tiles[(hp, m, qf)] = pu

            def alloc_ps_o(hp, qf):
                for hh in range(2):
                    ps_o[(hp, hh, qf)] = psp.tile(
                        [HD + 1, 512], FP32, name=f"pso{hp}{hh}{qf}",
                        tag="ps_acc", bufs=4)

            def emit_o(hp, m, qf):
                vd3 = vd_sb[m].rearrange("p (h e) -> p h e", e=HD + 1)
                pu = pu_tiles[(hp, m, qf)]
                for hh in range(2):
                    nc.tensor.matmul(
                        ps_o[(hp, hh, qf)][:],
                        lhsT=vd3[:, 2 * hp + hh, :],
                        rhs=pu[:, hh * 512:(hh + 1) * 512],
                        start=(m == 0), stop=(m == NKT - 1))

            def emit_norm(hp, qf):
                # o_sb[hp][0:64, qf half] = head 2hp, [64:128] = head 2hp+1,
                # each row block scaled by its softmax reciprocal. The
                # reciprocal reads the accumulator's ones-row straight from
                # PSUM; Pool broadcasts it across the 64 head partitions.
                cols = slice(qf * 512, (qf + 1) * 512)
                recs = []
                for hh in range(2):
                    dn = sm.tile([1, 512], FP32, name=f"dn{hp}{hh}{qf}",
                                 tag="dn", bufs=4)
                    nc.vector.tensor_copy(dn[:],
                                          ps_o[(hp, hh, qf)][HD:HD + 1, :])
                    rec = sm.tile([1, 512], FP32, name=f"rec{hp}{hh}{qf}",
                                  tag="rec", bufs=4)
                    nc.vector.reciprocal_approx_fast(rec[:], dn[:])
                    recs.append(rec)
                for hh in range(2):
                    bc = sm.tile([HD, 512], FP32, name=f"bc{hp}{hh}{qf}",
                                 tag="bc", bufs=2)
                    nc.gpsimd.partition_broadcast(bc[:], recs[hh][:])
                    nc.vector.tensor_mul(o_sb[hp][hh * HD:(hh + 1) * HD, cols],
                                         ps_o[(hp, hh, qf)][0:HD, :],
                                         bc[:])

            def emit_wo(qf, pts=range(CT), ob_eng="scalar"):
                cols = slice(qf * 512, (qf + 1) * 512)
                for pt in pts:
                    ps = psp.tile([P, 512], FP32, name=f"psw{pt}{qf}",
                                  tag="ps_acc", bufs=4)
                    for hp in range(G):
                        nc.tensor.matmul(
                            ps[:],
                            lhsT=wo_sb[:, hp * C + pt * P:
                                       hp * C + (pt + 1) * P],
                            rhs=o_sb[hp][:, cols],
                            start=(hp == 0), stop=(hp == G - 1))
                    ob = sm.tile([P, 512], FP16, name=f"ob{pt}{qf}",
                                 tag="ob", bufs=2)
                    if ob_eng == "vector":
                        nc.vector.tensor_copy(ob[:], ps[:])
                    else:
                        nc.scalar.copy(ob[:], ps[:])
                    nc.sync.dma_start(out=out_d[pt * P:(pt + 1) * P, cols],
                                      in_=ob[:])

            # ---- emission schedule ----------------------------------------
            # Phase A: minimal prologue for the first S pair.
            emit_qt_chunk(0, 0)
            emit_kt_chunk(0, 0)

            # Phase B: hp0 spine (qf-outer: all qf0 units then all qf1), PE
            # backfill ordered by DMA arrival and need-by unit: KT(hp0) JIT
            # chunks, QT(hp0,qf1) before unit 8, QT/KT(hp1), V, Vd.
            backfill = ([lambda j=j: emit_kt_chunk(0, j) for j in (1, 2, 3)]
                        + [lambda: emit_qt_chunk(0, 1),
                           lambda: emit_qt_chunk(1, 0),
                           lambda: emit_qt_chunk(1, 1)]
                        + [lambda j=j: emit_kt_chunk(1, j) for j in range(4)]
                        + [lambda m=m: emit_v(m) for m in range(NKT)]
                        + [lambda m=m: emit_vd(m) for m in range(NKT)])
            steps = [1, 1, 1, 1, 2, 2, 2, 2, 2, 2, 2, 2, 2, 2, 1, 1]
            bi = 0
            for u, (qf, m) in enumerate([(qf, m) for qf in range(2)
                                         for m in range(NKT)]):
                emit_s(0, m, qf)
                for _ in range(steps[u]):
                    if bi < len(backfill):
                        backfill[bi]()
                        bi += 1
            while bi < len(backfill):
                backfill[bi]()
                bi += 1

            # Phase C: hp1 spine (qf-outer). PE backfill: O(hp0) qf-grouped
            # then norm(hp0); O(hp1) lags once its accumulators are free.
            alloc_ps_o(0, 0)
            alloc_ps_o(0, 1)
            o0_fill = ([(0, m, 0) for m in range(NKT)]
                       + [(0, m, 1) for m in range(NKT)])
            o0i = 0
            o1i = 0
            o1_alloc = [False, False]
            n_ready = 0
            norm0_done = False

            def drain_o1(limit):
                nonlocal o1i
                while o1i < min(limit, n_ready):
                    m, qf = o1i % NKT, o1i // NKT
                    if not o1_alloc[qf]:
                        alloc_ps_o(1, qf)
                        o1_alloc[qf] = True
                    emit_o(1, m, qf)
                    o1i += 1

            norm1q0_done = False
            for u, (qf, m) in enumerate([(qf, m) for qf in range(2)
                                         for m in range(NKT)]):
                emit_s(1, m, qf)
                n_ready = u + 1
                if o0i < len(o0_fill):
                    for _ in range(2):
                        if o0i < len(o0_fill):
                            emit_o(*o0_fill[o0i])
                            o0i += 1
                    if o0i == len(o0_fill) and not norm0_done:
                        emit_norm(0, 0)
                        emit_norm(0, 1)
                        norm0_done = True
                else:
                    drain_o1(min(n_ready - 2,
                                 o1i + (3 if o1i == 0 else 2)))
                if o1i >= 8 and not norm1q0_done:
                    emit_norm(1, 0)
                    norm1q0_done = True

            # Phase D: tail. Remaining O(hp1,qf1), Wo(qf0) first half under
            # norm(1,qf1), then the rest, fp16 copies + out DMA.
            n_ready = 16
            drain_o1(15)
            emit_wo(0, pts=(0, 1))
            drain_o1(16)
            emit_norm(1, 1)
            emit_wo(0, pts=(2, 3))
            emit_wo(1)

    nc.compile()
    return nc


# --------------------------------------------------------------------------
# public entry points
# --------------------------------------------------------------------------

def _prepare(inputs):
    q = np.ascontiguousarray(inputs["q_feat"], np.float32).reshape(B, C, N)
    kv = np.ascontiguousarray(inputs["kv_feat"], np.float32).reshape(B, C, N)
    offsets = np.asarray(inputs["offsets"], np.float32)
    sal = np.asarray(inputs["saliency_map"], np.float32).reshape(B, N)
    gamma = float(np.asarray(inputs["gamma_val"]))

    GTs = [_gather_T(offsets[b], 1.0 + gamma * sal[b]) for b in range(B)]

    # union band-sparsity pattern of the gather matmul across batches, so the
    # SPMD program is identical on every core
    chunks = []
    for m in range(NKT):
        for k in range(NKT):
            if any(GTs[b][k * P:(k + 1) * P, m * P:(m + 1) * P].any()
                   for b in range(B)):
                chunks.append((m, k))

    Wq = np.asarray(inputs["Wq"], np.float32) * (HD ** -0.5)
    Wk = np.asarray(inputs["Wk"], np.float32)
    Wv = np.asarray(inputs["Wv"], np.float32)
    Wo = np.asarray(inputs["Wo"], np.float32)

    def ptile(a):
        # [T*P, X] -> [P, T*X]: partition-tile-major columns
        t = a.shape[0] // P
        return np.ascontiguousarray(
            a.reshape(t, P, a.shape[1]).transpose(1, 0, 2).reshape(P, -1)
        ).astype(NP_BF16)

    in_maps = []
    for core in range(NCORES):
        b, g = core // G, core % G
        rows = slice(g * CG, (g + 1) * CG)
        gt_stack = np.stack([GTs[b][k * P:(k + 1) * P, m * P:(m + 1) * P]
                             for (m, k) in chunks])     # [nch, P, P]
        in_maps.append({
            "xq": ptile(q[b]),
            "xkv": ptile(kv[b]),
            "wqT": ptile(Wq[rows].T),
            "wkT": ptile(Wk[rows].T),
            "wvT": ptile(Wv[rows].T),
            "woT": ptile(Wo[:, rows].T),
            "gt": np.ascontiguousarray(
                gt_stack.transpose(1, 0, 2).reshape(P, -1)).astype(NP_BF16),
        })

    def assemble(results):
        out = np.empty((B, C, N), np.float32)
        for b in range(B):
            out[b] = (results[G * b]["outT"].astype(np.float32)
                      + results[G * b + 1]["outT"].astype(np.float32))
        return out.reshape(B, C, HH, WW)

    nc = _build_program(chunks)
    return nc, in_maps, assemble


def _needs_fallback(inputs):
    try:
        if tuple(np.shape(inputs["q_feat"])) != (B, C, HH, WW):
            return True
        for bias in ("bq", "bk", "bv", "bo"):
            if np.any(np.asarray(inputs[bias], np.float32) != 0.0):
                return True
    except Exception:
        return True
    return False


def kernel(**inputs) -> np.ndarray:
    if _needs_fallback(inputs):
        return _reference_numpy(**{k: np.asarray(v, np.float32)
                                   for k, v in inputs.items()})
    nc, in_maps, assemble = _prepare(inputs)
    res = run_bass_kernel_spmd(nc, in_maps, core_ids=list(range(NCORES)))
    return assemble(res.results)


def kernel_traced(trace_cores=(0,), **inputs):
    """Like kernel() but returns (output, exec_time_ns, trace_path)."""
    nc, in_maps, assemble = _prepare(inputs)
    res = run_bass_kernel_spmd(nc, in_maps, core_ids=list(range(NCORES)),
                               trace=True, trace_cores=list(trace_cores))
    trace_path = None
    if res.instructions_and_trace is not None:
        trace_path = res.instructions_and_trace[1]
    return assemble(res.results), res.exec_time_ns, trace_path
